# revision 1
# baseline (speedup 1.0000x reference)
"""Trainium2 Bass kernel for LoRACrossAttnProcessor.

Strategy:
- Host: fold LoRA (W_eff = W + up @ down, exact), pre-transpose X/E/W so all
  device matmuls contract over the partition dim with no on-chip transposes.
- Shard: data-parallel over batch, 2 batch items per core, 8 cores.
- Device (per core, all fp32r = fp32 rounded to 11-bit mantissa; fp32 PSUM):
    K.T = Wk_eff @ E.T   [1280, 154]   (both batches, N padded to 256)
    V   = E @ Wv_eff.T   [77, 1280]    (per batch, natural layout)
    Q.T = Wq_eff @ X.T   [1280, 1024]  (per batch)
    per (batch, head): scores.T = (K.T)_h.T-chunks @ (Q.T)_h  -> [77, 1024]
      exps = exp(scores.T * scale)  (ACT, fused scale)
      sumexp = ones.T @ exps (PE), recip (DVE), partition-broadcast (DMA)
      A.T_h = V_h.T @ exps via col-tiled matmuls, normalized by recip (DVE STT)
    O = A @ Wo_eff.T  [1024, 1280]  (natural layout, streamed out)
- Host: gather batches, add bo.
"""

import numpy as np
from contextlib import ExitStack

import concourse.bass as bass
import concourse.mybir as mybir
import concourse.tile as tile
from concourse import bacc
from concourse.bass_utils import run_bass_kernel_spmd

F32 = mybir.dt.float32
F32R = mybir.dt.float32r
AF = mybir.ActivationFunctionType
MULT = mybir.AluOpType.mult

H = 8
B, S, C = 16, 1024, 1280
SENC, CENC = 77, 1024
D = C // H  # 160
NCORES = 8
BPC = B // NCORES  # 2 batches per core
P = 128
NCI_Q = C // P  # 10 contraction tiles for Q/O proj
NCI_KV = CENC // P  # 8 contraction tiles for K/V proj
NCO = C // P  # 10 output-channel tiles
NST = S // 512  # 2 seq chunks of 512
EPAD = 256  # padded encoder column count (2*77 -> 256)
ATTN_SCALE = 1.0 / float(np.sqrt(D))
OCHUNKS = [(0, 512), (512, 512), (1024, 256)]


def head_chunks(h):
    """Split head h's channel range [160h, 160h+160) into PE-tile-aligned
    blocks: size in {32, 64, 128}, offset % size == 0 within a 128-tile.

    Returns [(tile, offset, size, local_d0)]."""
    out = []
    g0, g1 = D * h, D * (h + 1)
    g = g0
    while g < g1:
        t, off = divmod(g, P)
        rem = min(P - off, g1 - g)
        s = 128
        while s > rem or off % s != 0:
            s //= 2
        assert s >= 32
        out.append((t, off, s, g - g0))
        g += s
    return out


def aligned_ranges(r0, r1):
    """Decompose [r0, r1) (within one 128 tile) into blocks of size 32/64/128
    with offset % size == 0 (SBUF partition-access alignment rule)."""
    out = []
    g = r0
    while g < r1:
        s = 128
        while s > r1 - g or g % s != 0:
            s //= 2
        out.append((g, s))
        g += s
    return out


def build():
    nc = bacc.Bacc("TRN2", target_bir_lowering=False, debug=False)
    xt_d = nc.dram_tensor("xt", [BPC, C, S], F32, kind="ExternalInput")
    et_d = nc.dram_tensor("et", [CENC, EPAD], F32, kind="ExternalInput")
    wqt_d = nc.dram_tensor("wqt", [C, C], F32, kind="ExternalInput")
    wkt_d = nc.dram_tensor("wkt", [CENC, C], F32, kind="ExternalInput")
    wvt_d = nc.dram_tensor("wvt", [CENC, C], F32, kind="ExternalInput")
    wot_d = nc.dram_tensor("wot", [C, C], F32, kind="ExternalInput")
    out_d = nc.dram_tensor("out", [BPC, S, C], F32, kind="ExternalOutput")

    with tile.TileContext(nc) as tc, ExitStack() as ctx:
        big = ctx.enter_context(tc.tile_pool(name="big", bufs=3))
        wblk = ctx.enter_context(tc.tile_pool(name="wblk", bufs=2))
        raw = ctx.enter_context(tc.tile_pool(name="raw", bufs=2))
        persist = ctx.enter_context(tc.tile_pool(name="persist", bufs=1))
        expp = ctx.enter_context(tc.tile_pool(name="expp", bufs=2))
        smallp = ctx.enter_context(tc.tile_pool(name="smallp", bufs=2))
        stag = ctx.enter_context(tc.tile_pool(name="stag", bufs=2))
        psum = ctx.enter_context(tc.tile_pool(name="psum", bufs=7, space="PSUM"))

        rnd_engines = [nc.vector, nc.vector]

        # ---- constants ----
        ones77f = persist.tile([SENC, 1], F32, tag="ones77f")
        nc.vector.memset(ones77f, 1.0)
        ones77r = persist.tile([SENC, 1], F32R, tag="ones77r")
        nc.vector.tensor_copy(out=ones77r, in_=ones77f)
        zeros_f = persist.tile([P, 2 * SENC], F32, tag="zeros_f")
        nc.vector.memset(zeros_f, 0.0)

        # ---- load & round E.T  [1024, 256] -> et_r [128, 8, 256] ----
        et_r = persist.tile([P, NCI_KV, EPAD], F32R, tag="et")
        for ci in range(NCI_KV):
            rw = raw.tile([P, NCI_Q, P], F32, tag="raw")
            nc.sync.dma_start(
                out=rw[:, :2, :].rearrange("p a b -> p (a b)"),
                in_=et_d.ap()[ci * P : (ci + 1) * P, :],
            )
            rnd_engines[ci % 2].tensor_copy(
                out=et_r[:, ci, :], in_=rw[:, :2, :].rearrange("p a b -> p (a b)")
            )

        # ---- K.T projection (both batches): kt_r[t] = [128, 154] ----
        kt_r = []
        for t in range(NCO):
            blk = wblk.tile([P, NCI_Q, EPAD], F32R, tag="wblk")
            for ci in range(NCI_KV):
                rw = raw.tile([P, NCI_Q, P], F32, tag="raw")
                nc.sync.dma_start(
                    out=rw[:, 0, :],
                    in_=wkt_d.ap()[ci * P : (ci + 1) * P, t * P : (t + 1) * P],
                )
                rnd_engines[ci % 2].tensor_copy(
                    out=blk[:, ci, :P], in_=rw[:, 0, :]
                )
            ps = psum.tile([P, EPAD], F32, tag="ps")
            for ci in range(NCI_KV):
                nc.tensor.matmul(
                    ps,
                    blk[:, ci, :P],
                    et_r[:, ci, :],
                    start=(ci == 0),
                    stop=(ci == NCI_KV - 1),
                )
            # Two parity-masked K.T copies: even heads' rows in kte (odd rows
            # zero) and vice versa. Scores matmuls can then use full 128-row
            # base-0 tiles; zeros kill the other heads' contributions.
            # (Accumulating matmuls from different PE row-groups into one
            # PSUM crash at runtime, so per-head row-chunks are not usable.)
            kte = persist.tile([P, 2 * SENC], F32R, tag=f"kte{t}", name=f"kte{t}")
            kto = persist.tile([P, 2 * SENC], F32R, tag=f"kto{t}", name=f"kto{t}")
            nc.vector.tensor_copy(out=kte, in_=zeros_f[:, : 2 * SENC])
            nc.vector.tensor_copy(out=kto, in_=zeros_f[:, : 2 * SENC])
            for h in range(H):
                r0 = max(D * h, P * t)
                r1 = min(D * h + D, P * t + P)
                if r0 >= r1:
                    continue
                dst = kte if h % 2 == 0 else kto
                for o, s in aligned_ranges(r0 - P * t, r1 - P * t):
                    nc.vector.tensor_copy(
                        out=dst[o : o + s, :], in_=ps[o : o + s, : 2 * SENC]
                    )
            kt_r.append((kte, kto))

        # ---- V projection (per batch, natural layout): v_nat[b] [77, 1280] ----
        v_nat = []
        for b in range(BPC):
            v_nat.append(
                persist.tile([SENC, C], F32R, tag=f"vnat{b}", name=f"vnat{b}")
            )
        for cc in range(0, C, 256):
            blk = wblk.tile([P, NCI_Q, EPAD], F32R, tag="wblk")
            for ci in range(NCI_KV):
                rw = raw.tile([P, NCI_Q, P], F32, tag="raw")
                nc.sync.dma_start(
                    out=rw[:, :2, :].rearrange("p a b -> p (a b)"),
                    in_=wvt_d.ap()[ci * P : (ci + 1) * P, cc : cc + 256],
                )
                rnd_engines[ci % 2].tensor_copy(
                    out=blk[:, ci, :],
                    in_=rw[:, :2, :].rearrange("p a b -> p (a b)"),
                )
            for b in range(BPC):
                ps = psum.tile([SENC, 512], F32, tag="ps")
                for ci in range(NCI_KV):
                    nc.tensor.matmul(
                        ps[:, :256],
                        et_r[:, ci, b * SENC : (b + 1) * SENC],
                        blk[:, ci, :],
                        start=(ci == 0),
                        stop=(ci == NCI_KV - 1),
                    )
                nc.vector.tensor_copy(
                    out=v_nat[b][:, cc : cc + 256], in_=ps[:, :256]
                )

        # ---- load & round X.T per batch: xt_r[b] [128, 10, 1024] ----
        xt_r = [None] * BPC
        for b in range(BPC):
            xt_r[b] = big.tile([P, NCI_Q, S], F32R, tag="big", name=f"xt{b}")
            for ci in range(NCI_Q):
                rw = raw.tile([P, NCI_Q, P], F32, tag="raw")
                nc.sync.dma_start(
                    out=rw[:, :8, :].rearrange("p a b -> p (a b)"),
                    in_=xt_d.ap()[b, ci * P : (ci + 1) * P, :],
                )
                rnd_engines[ci % 2].tensor_copy(
                    out=xt_r[b][:, ci, :],
                    in_=rw[:, :8, :].rearrange("p a b -> p (a b)"),
                )

        # ---- Q.T projection, batch-major (Wq streamed per batch) ----
        qt_r = [None] * BPC
        for b in range(BPC):
            qt_r[b] = big.tile([P, NCO, S], F32R, tag="big", name=f"qt{b}")
            for co in range(NCO):
                blk = wblk.tile([P, NCI_Q, EPAD], F32R, tag="wblk")
                rwb = raw.tile([P, NCI_Q, P], F32, tag="raw")
                nc.sync.dma_start(
                    out=rwb,
                    in_=wqt_d.ap()[:, co * P : (co + 1) * P].rearrange(
                        "(ci p) c -> p ci c", p=P
                    ),
                )
                for ci in range(NCI_Q):
                    rnd_engines[ci % 2].tensor_copy(
                        out=blk[:, ci, :P], in_=rwb[:, ci, :]
                    )
                for st in range(NST):
                    ps = psum.tile([P, 512], F32, tag="ps")
                    for ci in range(NCI_Q):
                        nc.tensor.matmul(
                            ps,
                            blk[:, ci, :P],
                            xt_r[b][:, ci, st * 512 : st * 512 + 512],
                            start=(ci == 0),
                            stop=(ci == NCI_Q - 1),
                        )
                    nc.vector.tensor_copy(
                        out=qt_r[b][:, co, st * 512 : st * 512 + 512], in_=ps
                    )

        # ---- attention per (batch, head) -> at_r[b] [128, 10, 1024] ----
        at_r = [None] * BPC
        for b in range(BPC):
            at_r[b] = big.tile([P, NCO, S], F32R, tag="big", name=f"at{b}")
            for h in range(H):
                hch = head_chunks(h)
                for st in range(NST):
                    sl = slice(st * 512, st * 512 + 512)
                    # scores.T [77, 512]: full 128-row tiles of parity-masked
                    # K.T accumulated over the tiles this head touches.
                    tiles = sorted({t for (t, _, _, _) in hch})
                    ps_s = psum.tile([SENC, 512], F32, tag="ps")
                    for i, t in enumerate(tiles):
                        nc.tensor.matmul(
                            ps_s,
                            kt_r[t][h % 2][:, b * SENC : (b + 1) * SENC],
                            qt_r[b][:, t, sl],
                            start=(i == 0),
                            stop=(i == len(tiles) - 1),
                        )
                    exps = expp.tile([SENC, 512], F32R, tag="exps")
                    nc.scalar.activation(
                        out=exps, in_=ps_s, func=AF.Exp, scale=ATTN_SCALE
                    )
                    # sumexp [1, 512] on PE; reciprocal; partition-broadcast
                    ps_se = psum.tile([1, 512], F32, tag="ps")
                    nc.tensor.matmul(ps_se, ones77r, exps, start=True, stop=True)
                    rec = smallp.tile([1, 512], F32, tag="rec")
                    nc.vector.reciprocal(out=rec, in_=ps_se)
                    bc = smallp.tile([P, 512], F32, tag="bc")
                    nc.gpsimd.partition_broadcast(bc, rec)
                    # A.T_h = V_h.T @ exps, landed at global partition offsets
                    # via col-tiling; normalize by bc while copying to SBUF.
                    for t, off, size, l0 in hch:
                        ps_av = psum.tile([P, 512], F32, tag="ps")
                        nc.tensor.matmul(
                            ps_av[0:size, :],
                            v_nat[b][:, D * h + l0 : D * h + l0 + size],
                            exps,
                            start=True,
                            stop=True,
                        )
                        avt = smallp.tile([P, 512], F32R, tag="avt")
                        nc.vector.scalar_tensor_tensor(
                            out=avt[0:size, :],
                            in0=ps_av[0:size, :],
                            scalar=1.0,
                            in1=bc[0:size, :],
                            op0=MULT,
                            op1=MULT,
                        )
                        # fp32r matmuls can't target PSUM partition offsets;
                        # DMA does the partition shift into the assembled A.T.
                        nc.sync.dma_start(
                            out=at_r[b][off : off + size, t, sl],
                            in_=avt[0:size, :],
                        )

        # ---- O projection as O.T (Wo streamed once, stage-major) ----
        # O.T[co, m] = sum_ch Wo_eff[co, ch] A[m, ch]; DMA writes DRAM with a
        # transposed access pattern (partition dim -> channel, 4B stride).
        for co in range(NCO):
            blk = wblk.tile([P, NCI_Q, EPAD], F32R, tag="wblk")
            rwb = raw.tile([P, NCI_Q, P], F32, tag="raw")
            nc.sync.dma_start(
                out=rwb,
                in_=wot_d.ap()[:, co * P : (co + 1) * P].rearrange(
                    "(ci p) c -> p ci c", p=P
                ),
            )
            for ci in range(NCI_Q):
                rnd_engines[ci % 2].tensor_copy(
                    out=blk[:, ci, :P], in_=rwb[:, ci, :]
                )
            for b in range(BPC):
                for st in range(NST):
                    ps = psum.tile([P, 512], F32, tag="ps")
                    for ci in range(NCI_Q):
                        nc.tensor.matmul(
                            ps,
                            blk[:, ci, :P],
                            at_r[b][:, ci, st * 512 : st * 512 + 512],
                            start=(ci == 0),
                            stop=(ci == NCI_Q - 1),
                        )
                    ot = stag.tile([P, 512], F32, tag="ot")
                    nc.scalar.copy(out=ot, in_=ps)
                    nc.sync.dma_start(
                        out=out_d.ap()[
                            b, st * 512 : st * 512 + 512, co * P : (co + 1) * P
                        ].rearrange("s c -> c s"),
                        in_=ot,
                    )

    nc.compile()
    return nc


_NC_CACHE = []


def _get_nc():
    if not _NC_CACHE:
        _NC_CACHE.append(build())
    return _NC_CACHE[0]


def make_in_maps(hidden_states, encoder_hidden_states, Wq, Wk, Wv, Wo,
                 q_down, q_up, k_down, k_up, v_down, v_up, o_down, o_up):
    wq = (Wq.astype(np.float64) + q_up.astype(np.float64) @ q_down.astype(np.float64))
    wk = (Wk.astype(np.float64) + k_up.astype(np.float64) @ k_down.astype(np.float64))
    wv = (Wv.astype(np.float64) + v_up.astype(np.float64) @ v_down.astype(np.float64))
    wo = (Wo.astype(np.float64) + o_up.astype(np.float64) @ o_down.astype(np.float64))
    wqt = np.ascontiguousarray(wq.T.astype(np.float32))
    wkt = np.ascontiguousarray(wk.T.astype(np.float32))
    wvt = np.ascontiguousarray(wv.T.astype(np.float32))
    wot = np.ascontiguousarray(wo.T.astype(np.float32))

    in_maps = []
    for c in range(NCORES):
        hs = hidden_states[c * BPC : (c + 1) * BPC]  # [2, S, C]
        xt = np.ascontiguousarray(hs.transpose(0, 2, 1).astype(np.float32))
        enc = encoder_hidden_states[c * BPC : (c + 1) * BPC]  # [2, 77, 1024]
        et = np.zeros((CENC, EPAD), np.float32)
        for b in range(BPC):
            et[:, b * SENC : (b + 1) * SENC] = enc[b].T
        in_maps.append(
            {"xt": xt, "et": et, "wqt": wqt, "wkt": wkt, "wvt": wvt, "wot": wot}
        )
    return in_maps


def kernel(hidden_states, encoder_hidden_states, Wq, Wk, Wv, Wo, bo,
           q_down, q_up, k_down, k_up, v_down, v_up, o_down, o_up):
    nc = _get_nc()
    in_maps = make_in_maps(
        hidden_states, encoder_hidden_states, Wq, Wk, Wv, Wo,
        q_down, q_up, k_down, k_up, v_down, v_up, o_down, o_up,
    )
    res = run_bass_kernel_spmd(nc, in_maps, list(range(NCORES)))
    out = np.concatenate([res.results[c]["out"] for c in range(NCORES)], axis=0)
    out = out + bo.astype(np.float32)[None, None, :]
    return out.astype(np.float32)



# revision 2
# speedup vs baseline: 21.6558x; 21.6558x over previous
"""Trainium2 Bass kernel for LoRACrossAttnProcessor.

Strategy (v2 — bf16, descriptor-friendly DMA):
- Host: fold LoRA (W_eff = W + up @ down, exact, f64), permute the QKV output
  channels so head h's first 128 channels form tile h and the 8 heads'
  32-channel remainders pack into tiles 8/9 (32-aligned). Wo's input side gets
  the same permutation, so the device never needs unaligned head slicing.
  All inputs shipped as bf16 (tol 2e-2; measured quantization err ~7e-3).
- Shard: data-parallel over batch, 2 batch items per core, 8 cores.
- Device (per core):
    K.T tiles = Wk_blk @ E.T       [128, 154] x8 full + x8 per-head masked
    V   = E @ Wv_blk               [77, 1280]  (natural, permuted cols)
    Q.T = Wq_blk @ X.T             [128, 10, 1024] per batch
    per (batch, head, s-chunk 512):
      scores.T = kt[h].T-acc @ Q.T chunks    [77, 512]
      exps = exp(scores.T * scale)           (ACT, fused scale)
      sumexp = ones.T @ exps (PE), recip (DVE), partition-broadcast (GpSimd)
      A.T[h-tile]  = V_h.T @ exps, normalized via DVE STT (direct, offset 0)
      A.T[rem-32]  = V_rem.T @ exps, STT + SBUF->SBUF DMA partition shift
    O = A.T-slices.T @ Wo_blk      [128 s, 512 c] natural rows -> contiguous
                                   2KB-per-partition DRAM writes
- Host: gather batches, add bo.
"""

import numpy as np
from contextlib import ExitStack

import ml_dtypes

import concourse.bass as bass
import concourse.mybir as mybir
import concourse.tile as tile
from concourse import bacc
from concourse.bass_utils import run_bass_kernel_spmd

F32 = mybir.dt.float32
BF16 = mybir.dt.bfloat16
AF = mybir.ActivationFunctionType
MULT = mybir.AluOpType.mult

H = 8
B, S, C = 16, 1024, 1280
SENC, CENC = 77, 1024
D = C // H  # 160
NCORES = 8
BPC = B // NCORES  # 2 batches per core
P = 128
NCI_Q = C // P  # 10 contraction tiles for Q/O proj
NCI_KV = CENC // P  # 8 contraction tiles for K/V proj
NCO = C // P  # 10 output-channel tiles
NST = S // 512  # 2 seq chunks of 512
EPAD = 256  # padded encoder column count (2*77 -> 256)
SE2 = 2 * SENC  # 154
ATTN_SCALE = 1.0 / float(np.sqrt(D))
CBLOCKS = [(0, 512), (512, 512), (1024, 256)]  # column blocks for streamed W


def make_perm():
    """new channel -> old channel. Tiles 0..7 = head h channels [0,128);
    tiles 8,9 = the 8 heads' remainder channels [128,160), 32-aligned."""
    perm = np.zeros(C, np.int64)
    for h in range(H):
        perm[h * P : (h + 1) * P] = h * D + np.arange(P)
    for h in range(H):
        perm[8 * P + h * 32 : 8 * P + (h + 1) * 32] = h * D + P + np.arange(32)
    return perm


PERM = make_perm()


def build():
    nc = bacc.Bacc("TRN2", target_bir_lowering=False, debug=False)
    xt_d = nc.dram_tensor("xt", [BPC, C, S], BF16, kind="ExternalInput")
    et_d = nc.dram_tensor("et", [CENC, EPAD], BF16, kind="ExternalInput")
    wqt_d = nc.dram_tensor("wqt", [C, C], BF16, kind="ExternalInput")
    wkt_d = nc.dram_tensor("wkt", [CENC, C], BF16, kind="ExternalInput")
    wvt_d = nc.dram_tensor("wvt", [CENC, C], BF16, kind="ExternalInput")
    wot_d = nc.dram_tensor("wot", [C, C], BF16, kind="ExternalInput")
    out_d = nc.dram_tensor("out", [BPC, S, C], F32, kind="ExternalOutput")

    with tile.TileContext(nc) as tc, ExitStack() as ctx:
        xpool = ctx.enter_context(tc.tile_pool(name="xpool", bufs=2))
        qa = ctx.enter_context(tc.tile_pool(name="qa", bufs=4))
        wblk = ctx.enter_context(tc.tile_pool(name="wblk", bufs=4))
        persist = ctx.enter_context(tc.tile_pool(name="persist", bufs=1))
        expp = ctx.enter_context(tc.tile_pool(name="expp", bufs=4))
        smallp = ctx.enter_context(tc.tile_pool(name="smallp", bufs=2))
        ostag = ctx.enter_context(tc.tile_pool(name="ostag", bufs=4))
        ps_qo = ctx.enter_context(tc.tile_pool(name="psqo", bufs=2, space="PSUM"))
        ps_s = ctx.enter_context(tc.tile_pool(name="pss", bufs=2, space="PSUM"))
        ps_se = ctx.enter_context(tc.tile_pool(name="psse", bufs=1, space="PSUM"))
        ps_av = ctx.enter_context(tc.tile_pool(name="psav", bufs=2, space="PSUM"))
        ps_av2 = ctx.enter_context(tc.tile_pool(name="psav2", bufs=1, space="PSUM"))

        # ---- constants ----
        ones77 = persist.tile([SENC, 1], BF16, tag="ones77")
        nc.vector.memset(ones77, 1.0)

        # ---- bulk input loads (one DMA each, contiguous 1-4KB descriptors) ----
        et_r = persist.tile([P, NCI_KV, EPAD], BF16, tag="et")
        nc.sync.dma_start(
            out=et_r, in_=et_d.ap().rearrange("(ci p) e -> p ci e", p=P)
        )
        xt_r = []
        for b in range(BPC):
            t = xpool.tile([P, NCI_Q, S], BF16, tag="xt", name=f"xt{b}")
            nc.sync.dma_start(
                out=t, in_=xt_d.ap()[b].rearrange("(ci p) s -> p ci s", p=P)
            )
            xt_r.append(t)

        # ---- K.T projection: kt[0..7] full tiles, ktm[0..7] masked remainders ----
        kt = [None] * H
        ktm = [None] * H
        wk_blocks = []
        for bi, (cc, w) in enumerate(CBLOCKS):
            blk = wblk.tile([P, NCI_KV, 512], BF16, tag="wblk", name=f"wk{bi}")
            nc.sync.dma_start(
                out=blk[:, :, :w],
                in_=wkt_d.ap()[:, cc : cc + w].rearrange("(ci p) c -> p ci c", p=P),
            )
            wk_blocks.append(blk)
        for co in range(NCO):
            bi = min(co // 4, 2)
            cc, w = CBLOCKS[bi]
            cl = co * P - cc
            blk = wk_blocks[bi]
            ps = ps_qo.tile([P, EPAD], F32, tag="ps")
            for ci in range(NCI_KV):
                nc.tensor.matmul(
                    ps,
                    blk[:, ci, cl : cl + P],
                    et_r[:, ci, :],
                    start=(ci == 0),
                    stop=(ci == NCI_KV - 1),
                )
            if co < 8:
                t = persist.tile([P, SE2], BF16, tag=f"kt{co}", name=f"kt{co}")
                nc.vector.tensor_copy(out=t, in_=ps[:, :SE2])
                kt[co] = t
            else:
                for j in range(4):
                    h = (co - 8) * 4 + j
                    t = persist.tile([P, SE2], BF16, tag=f"ktm{h}", name=f"ktm{h}")
                    nc.vector.memset(t, 0.0)
                    nc.vector.tensor_copy(
                        out=t[32 * j : 32 * j + 32, :],
                        in_=ps[32 * j : 32 * j + 32, :SE2],
                    )
                    ktm[h] = t

        # ---- V projection (natural layout, permuted cols): v_nat[b] [77, 1280] ----
        v_nat = [
            persist.tile([SENC, C], BF16, tag=f"vnat{b}", name=f"vnat{b}")
            for b in range(BPC)
        ]
        for bi, (cc, w) in enumerate(CBLOCKS):
            blk = wblk.tile([P, NCI_KV, 512], BF16, tag="wblk", name=f"wv{bi}")
            nc.sync.dma_start(
                out=blk[:, :, :w],
                in_=wvt_d.ap()[:, cc : cc + w].rearrange("(ci p) c -> p ci c", p=P),
            )
            for b in range(BPC):
                ps = ps_s.tile([SENC, 512], F32, tag="ps")
                for ci in range(NCI_KV):
                    nc.tensor.matmul(
                        ps[:, :w],
                        et_r[:, ci, b * SENC : (b + 1) * SENC],
                        blk[:, ci, :w],
                        start=(ci == 0),
                        stop=(ci == NCI_KV - 1),
                    )
                nc.vector.tensor_copy(out=v_nat[b][:, cc : cc + w], in_=ps[:, :w])

        # ---- Q.T projection: qt[b] [128, 10, 1024] ----
        qt_r = [
            qa.tile([P, NCO, S], BF16, tag="qa", name=f"qt{b}") for b in range(BPC)
        ]
        for bi, (cc, w) in enumerate(CBLOCKS):
            blk = wblk.tile([P, NCI_Q, 512], BF16, tag="wblk", name=f"wq{bi}")
            nc.sync.dma_start(
                out=blk[:, :, :w],
                in_=wqt_d.ap()[:, cc : cc + w].rearrange("(ci p) c -> p ci c", p=P),
            )
            for b in range(BPC):
                for col in range(0, w, P):
                    co = (cc + col) // P
                    for st in range(NST):
                        ps = ps_qo.tile([P, 512], F32, tag="ps")
                        for ci in range(NCI_Q):
                            nc.tensor.matmul(
                                ps,
                                blk[:, ci, col : col + P],
                                xt_r[b][:, ci, st * 512 : st * 512 + 512],
                                start=(ci == 0),
                                stop=(ci == NCI_Q - 1),
                            )
                        nc.vector.tensor_copy(
                            out=qt_r[b][:, co, st * 512 : st * 512 + 512], in_=ps
                        )

        # ---- attention per (batch, head, s-chunk) -> at[b] [128, 10, 1024] ----
        at_r = [
            qa.tile([P, NCO, S], BF16, tag="qa", name=f"at{b}") for b in range(BPC)
        ]
        for b in range(BPC):
            bs = slice(b * SENC, (b + 1) * SENC)
            for h in range(H):
                for st in range(NST):
                    sl = slice(st * 512, st * 512 + 512)
                    pss = ps_s.tile([SENC, 512], F32, tag="ps")
                    nc.tensor.matmul(
                        pss, kt[h][:, bs], qt_r[b][:, h, sl], start=True, stop=False
                    )
                    nc.tensor.matmul(
                        pss,
                        ktm[h][:, bs],
                        qt_r[b][:, 8 + h // 4, sl],
                        start=False,
                        stop=True,
                    )
                    exps = expp.tile([SENC, 512], BF16, tag="exps")
                    nc.scalar.activation(
                        out=exps, in_=pss, func=AF.Exp, scale=ATTN_SCALE
                    )
                    pse = ps_se.tile([1, 512], F32, tag="se")
                    nc.tensor.matmul(pse, ones77, exps, start=True, stop=True)
                    rec = smallp.tile([1, 512], F32, tag="rec")
                    nc.vector.reciprocal(out=rec, in_=pse)
                    bc = smallp.tile([P, 512], F32, tag="bc")
                    nc.gpsimd.partition_broadcast(bc, rec)
                    # head-major 128 channels -> tile h at offset 0, no shift
                    pav = ps_av.tile([P, 512], F32, tag="av")
                    nc.tensor.matmul(
                        pav,
                        v_nat[b][:, h * P : (h + 1) * P],
                        exps,
                        start=True,
                        stop=True,
                    )
                    nc.vector.scalar_tensor_tensor(
                        out=at_r[b][:, h, sl],
                        in0=pav,
                        scalar=1.0,
                        in1=bc,
                        op0=MULT,
                        op1=MULT,
                    )
                    # 32-channel remainder -> tile 8/9 via DMA partition shift
                    pav2 = ps_av2.tile([32, 512], F32, tag="av2")
                    nc.tensor.matmul(
                        pav2,
                        v_nat[b][:, 8 * P + 32 * h : 8 * P + 32 * h + 32],
                        exps,
                        start=True,
                        stop=True,
                    )
                    avt = smallp.tile([32, 512], BF16, tag="avt")
                    nc.vector.scalar_tensor_tensor(
                        out=avt,
                        in0=pav2,
                        scalar=1.0,
                        in1=bc[0:32, :],
                        op0=MULT,
                        op1=MULT,
                    )
                    off = 32 * (h % 4)
                    nc.sync.dma_start(
                        out=at_r[b][off : off + 32, 8 + h // 4, sl], in_=avt
                    )

        # ---- O projection, natural rows: out[s, c] (contiguous DRAM writes) ----
        for bi, (cc, w) in enumerate(CBLOCKS):
            blk = wblk.tile([P, NCI_Q, 512], BF16, tag="wblk", name=f"wo{bi}")
            nc.sync.dma_start(
                out=blk[:, :, :w],
                in_=wot_d.ap()[:, cc : cc + w].rearrange("(ci p) c -> p ci c", p=P),
            )
            for b in range(BPC):
                for sc in range(S // P):
                    ps = ps_qo.tile([P, 512], F32, tag="ps")
                    for ci in range(NCI_Q):
                        nc.tensor.matmul(
                            ps[:, :w],
                            at_r[b][:, ci, sc * P : (sc + 1) * P],
                            blk[:, ci, :w],
                            start=(ci == 0),
                            stop=(ci == NCI_Q - 1),
                        )
                    ot = ostag.tile([P, 512], F32, tag="ot")
                    nc.scalar.copy(out=ot[:, :w], in_=ps[:, :w])
                    nc.sync.dma_start(
                        out=out_d.ap()[b, sc * P : (sc + 1) * P, cc : cc + w],
                        in_=ot[:, :w],
                    )

    nc.compile()
    return nc


_NC_CACHE = []


def _get_nc():
    if not _NC_CACHE:
        _NC_CACHE.append(build())
    return _NC_CACHE[0]


def make_in_maps(hidden_states, encoder_hidden_states, Wq, Wk, Wv, Wo,
                 q_down, q_up, k_down, k_up, v_down, v_up, o_down, o_up):
    bf = ml_dtypes.bfloat16
    wq = Wq.astype(np.float64) + q_up.astype(np.float64) @ q_down.astype(np.float64)
    wk = Wk.astype(np.float64) + k_up.astype(np.float64) @ k_down.astype(np.float64)
    wv = Wv.astype(np.float64) + v_up.astype(np.float64) @ v_down.astype(np.float64)
    wo = Wo.astype(np.float64) + o_up.astype(np.float64) @ o_down.astype(np.float64)
    # permute QKV output channels / Wo input channels to head-tile layout
    wqt = np.ascontiguousarray(wq[PERM, :].T.astype(bf))
    wkt = np.ascontiguousarray(wk[PERM, :].T.astype(bf))
    wvt = np.ascontiguousarray(wv[PERM, :].T.astype(bf))
    wot = np.ascontiguousarray(wo[:, PERM].T.astype(bf))

    in_maps = []
    for c in range(NCORES):
        hs = hidden_states[c * BPC : (c + 1) * BPC]  # [2, S, C]
        xt = np.ascontiguousarray(hs.transpose(0, 2, 1)).astype(bf)
        enc = encoder_hidden_states[c * BPC : (c + 1) * BPC]  # [2, 77, 1024]
        et = np.zeros((CENC, EPAD), bf)
        for b in range(BPC):
            et[:, b * SENC : (b + 1) * SENC] = enc[b].T.astype(bf)
        in_maps.append(
            {"xt": xt, "et": et, "wqt": wqt, "wkt": wkt, "wvt": wvt, "wot": wot}
        )
    return in_maps


def kernel(hidden_states, encoder_hidden_states, Wq, Wk, Wv, Wo, bo,
           q_down, q_up, k_down, k_up, v_down, v_up, o_down, o_up):
    nc = _get_nc()
    in_maps = make_in_maps(
        hidden_states, encoder_hidden_states, Wq, Wk, Wv, Wo,
        q_down, q_up, k_down, k_up, v_down, v_up, o_down, o_up,
    )
    res = run_bass_kernel_spmd(nc, in_maps, list(range(NCORES)))
    out = np.concatenate([res.results[c]["out"] for c in range(NCORES)], axis=0)
    out = out + bo.astype(np.float32)[None, None, :]
    return out.astype(np.float32)


# revision 25
# speedup vs baseline: 30.0619x; 1.3882x over previous
"""Trainium2 Bass kernel for LoRACrossAttnProcessor.

Strategy (v2 — bf16, descriptor-friendly DMA):
- Host: fold LoRA (W_eff = W + up @ down, exact, f64), permute the QKV output
  channels so head h's first 128 channels form tile h and the 8 heads'
  32-channel remainders pack into tiles 8/9 (32-aligned). Wo's input side gets
  the same permutation, so the device never needs unaligned head slicing.
  All inputs shipped as bf16 (tol 2e-2; measured quantization err ~7e-3).
- Shard: data-parallel over batch, 2 batch items per core, 8 cores.
- Device (per core):
    K.T tiles = Wk_blk @ E.T       [128, 154] x8 full + x8 per-head masked
    V   = E @ Wv_blk               [77, 1280]  (natural, permuted cols)
    Q.T = Wq_blk @ X.T             [128, 10, 1024] per batch
    per (batch, head, s-chunk 512):
      scores.T = kt[h].T-acc @ Q.T chunks    [77, 512]
      exps = exp(scores.T * scale)           (ACT, fused scale)
      sumexp = ones.T @ exps (PE), recip (DVE), partition-broadcast (GpSimd)
      A.T[h-tile]  = V_h.T @ exps, normalized via DVE STT (direct, offset 0)
      A.T[rem-32]  = V_rem.T @ exps, STT + SBUF->SBUF DMA partition shift
    O = A.T-slices.T @ Wo_blk      [128 s, 512 c] natural rows -> contiguous
                                   2KB-per-partition DRAM writes
- Host: gather batches, add bo.
"""

import numpy as np
from contextlib import ExitStack

import ml_dtypes

import concourse.bass as bass
import concourse.mybir as mybir
import concourse.tile as tile
from concourse import bacc
from concourse.bass_utils import run_bass_kernel_spmd

F32 = mybir.dt.float32
BF16 = mybir.dt.bfloat16
AF = mybir.ActivationFunctionType
MULT = mybir.AluOpType.mult

H = 8
B, S, C = 16, 1024, 1280
SENC, CENC = 77, 1024
D = C // H  # 160
NCORES = 8
BPC = B // NCORES  # 2 batches per core
P = 128
NCI_Q = C // P  # 10 contraction tiles for Q/O proj
NCI_KV = CENC // P  # 8 contraction tiles for K/V proj
NCO = C // P  # 10 output-channel tiles
NST = S // 512  # 2 seq chunks of 512
EPAD = 160  # padded encoder column count (2*77 -> 160, 32-aligned)
SE2 = 2 * SENC  # 154
ATTN_SCALE = 1.0 / float(np.sqrt(D))
CBLOCKS = [(0, 512), (512, 512), (1024, 256)]  # column blocks for streamed W


def make_perm():
    """new channel -> old channel. Tiles 0..7 = head h channels [0,128);
    tiles 8,9 = the 8 heads' remainder channels [128,160), 32-aligned."""
    perm = np.zeros(C, np.int64)
    for h in range(H):
        perm[h * P : (h + 1) * P] = h * D + np.arange(P)
    for h in range(H):
        perm[8 * P + h * 32 : 8 * P + (h + 1) * 32] = h * D + P + np.arange(32)
    return perm


PERM = make_perm()


def build():
    nc = bacc.Bacc("TRN2", target_bir_lowering=False, debug=False)
    xt_d = nc.dram_tensor("xt", [BPC, C, S], BF16, kind="ExternalInput")
    et_d = nc.dram_tensor("et", [CENC, EPAD], BF16, kind="ExternalInput")
    wqt_d = nc.dram_tensor("wqt", [C, C], BF16, kind="ExternalInput")
    wkt_d = nc.dram_tensor("wkt", [CENC, C], BF16, kind="ExternalInput")
    wvt_d = nc.dram_tensor("wvt", [CENC, C], BF16, kind="ExternalInput")
    wot_d = nc.dram_tensor("wot", [C, C], BF16, kind="ExternalInput")
    out_d = nc.dram_tensor("out", [BPC, S, C], F32, kind="ExternalOutput")

    with tile.TileContext(nc) as tc, ExitStack() as ctx:
        xpool = ctx.enter_context(tc.tile_pool(name="xpool", bufs=2))
        qa = ctx.enter_context(tc.tile_pool(name="qa", bufs=4))
        wblk = ctx.enter_context(tc.tile_pool(name="wblk", bufs=3))
        persist = ctx.enter_context(tc.tile_pool(name="persist", bufs=1))
        expp = ctx.enter_context(tc.tile_pool(name="expp", bufs=6))
        smallp = ctx.enter_context(tc.tile_pool(name="smallp", bufs=4))
        ostag = ctx.enter_context(tc.tile_pool(name="ostag", bufs=3))
        ps_qo = ctx.enter_context(tc.tile_pool(name="psqo", bufs=2, space="PSUM"))
        ps_s = ctx.enter_context(tc.tile_pool(name="pss", bufs=2, space="PSUM"))
        ps_av = ctx.enter_context(tc.tile_pool(name="psav", bufs=2, space="PSUM"))
        ps_av2 = ctx.enter_context(tc.tile_pool(name="psav2", bufs=2, space="PSUM"))

        # warm the ACT exp table during the initial DMA wait (first-use
        # ACT_TABLE_LOAD costs ~2.7us; pull it off the attention ramp)
        warm = persist.tile([1, 8], F32, tag="warm")
        nc.vector.memset(warm, 0.0)
        nc.scalar.activation(out=warm, in_=warm, func=AF.Exp, scale=1.0)
        # dense dummy matmuls during the initial DMA wait: HAM un-throttles
        # the PE clock (1.2->2.4 GHz) after ~3.4us of sustained busy, so the
        # first real (DMA-paced) matmuls run warm instead of at half clock
        warm_s = persist.tile([P, 16], BF16, tag="warm_s")
        warm_m = persist.tile([P, 512], BF16, tag="warm_m")
        nc.vector.memset(warm_s, 0.0)
        nc.vector.memset(warm_m, 0.0)
        ps_w = ps_qo.tile([16, 512], F32, tag="ps", name="warm_ps")
        NWARM = 10
        for i in range(NWARM):
            nc.tensor.matmul(
                ps_w, warm_s, warm_m, start=(i == 0), stop=(i == NWARM - 1)
            )
        nc.vector.tensor_copy(out=warm_s[0:16, 0:8], in_=ps_w[:, 0:8])

        # ---- bulk input loads (one DMA each, contiguous 1-4KB descriptors) ----
        # issue order: K-proj inputs first so the PE can start ASAP
        et_r = persist.tile([P, NCI_KV, EPAD], BF16, tag="et")
        for ci in range(NCI_KV):
            nc.sync.dma_start(
                out=et_r[:, ci, :], in_=et_d.ap()[ci * P : (ci + 1) * P, :]
            )
        kt = [None] * H
        ktm = [None] * H
        wk_blocks = []
        for bi, (cc, w) in enumerate(CBLOCKS):
            blk = wblk.tile([P, NCI_KV, 512], BF16, tag="wblk", name=f"wk{bi}")
            if bi == 0:
                for ci in range(NCI_KV):
                    nc.sync.dma_start(
                        out=blk[:, ci, :w],
                        in_=wkt_d.ap()[ci * P : (ci + 1) * P, cc : cc + w],
                    )
            else:
                nc.sync.dma_start(
                    out=blk[:, :, :w],
                    in_=wkt_d.ap()[:, cc : cc + w].rearrange(
                        "(ci p) c -> p ci c", p=P
                    ),
                )
            wk_blocks.append(blk)
        # first Q-weight block preloaded into a dedicated tile so Q proj can
        # start the moment K/V proj drain (wblk slots stay free for wv/wq1/2)
        wq0 = persist.tile([P, NCI_Q, 512], BF16, tag="wq0")
        nc.sync.dma_start(
            out=wq0, in_=wqt_d.ap()[:, 0:512].rearrange("(ci p) c -> p ci c", p=P)
        )
        xt_r = []
        for b in range(BPC):
            t = xpool.tile([P, NCI_Q, S], BF16, tag="xt", name=f"xt{b}")
            for ci in range(NCI_Q):
                nc.sync.dma_start(
                    out=t[:, ci, :], in_=xt_d.ap()[b, ci * P : (ci + 1) * P, :]
                )
            xt_r.append(t)

        # ---- K.T projection: kt[0..7] full tiles, ktm[0..7] masked remainders ----
        for co in range(NCO):
            bi = min(co // 4, 2)
            cc, w = CBLOCKS[bi]
            cl = co * P - cc
            blk = wk_blocks[bi]
            ps = ps_qo.tile([P, EPAD], F32, tag="ps")
            for ci in range(NCI_KV):
                nc.tensor.matmul(
                    ps,
                    blk[:, ci, cl : cl + P],
                    et_r[:, ci, :],
                    start=(ci == 0),
                    stop=(ci == NCI_KV - 1),
                )
            if co < 8:
                t = persist.tile([P, SE2], BF16, tag=f"kt{co}", name=f"kt{co}")
                nc.vector.tensor_copy(out=t, in_=ps[:, :SE2])
                kt[co] = t
            else:
                for j in range(4):
                    h = (co - 8) * 4 + j
                    t = persist.tile([P, SE2], BF16, tag=f"ktm{h}", name=f"ktm{h}")
                    nc.vector.memset(t, 0.0)
                    nc.vector.tensor_copy(
                        out=t[32 * j : 32 * j + 32, :],
                        in_=ps[32 * j : 32 * j + 32, :SE2],
                    )
                    ktm[h] = t

        # ---- V projection (natural layout, permuted cols) ----
        # v_nat[b] [77, 1024]: heads' main 128-channel chunks.
        # v2e[b][h] [77, 64]: ones column at 0, head h's 32 remainder channels
        # at cols 32:64, so the AV-remainder matmul emits sumexp at output row
        # 0 and the remainder at partition base 32 (DVE bases must be
        # 32-aligned; only DMA may shift partitions).
        v_nat = [
            persist.tile([SENC, 8 * P], BF16, tag=f"vnat{b}", name=f"vnat{b}")
            for b in range(BPC)
        ]
        v2e = [
            [
                persist.tile([SENC, 64], BF16, tag=f"v2e{b}_{h}", name=f"v2e{b}_{h}")
                for h in range(H)
            ]
            for b in range(BPC)
        ]
        for b in range(BPC):
            for h in range(H):
                nc.vector.memset(v2e[b][h], 1.0)
        for bi, (cc, w) in enumerate(CBLOCKS):
            blk = wblk.tile([P, NCI_KV, 512], BF16, tag="wblk", name=f"wv{bi}")
            nc.sync.dma_start(
                out=blk[:, :, :w],
                in_=wvt_d.ap()[:, cc : cc + w].rearrange("(ci p) c -> p ci c", p=P),
            )
            for b in range(BPC):
                ps = ps_s.tile([SENC, 512], F32, tag="ps")
                for ci in range(NCI_KV):
                    nc.tensor.matmul(
                        ps[:, :w],
                        et_r[:, ci, b * SENC : (b + 1) * SENC],
                        blk[:, ci, :w],
                        start=(ci == 0),
                        stop=(ci == NCI_KV - 1),
                    )
                if bi < 2:
                    nc.vector.tensor_copy(out=v_nat[b][:, cc : cc + w], in_=ps[:, :w])
                else:
                    for h in range(H):
                        nc.vector.tensor_copy(
                            out=v2e[b][h][:, 32:64], in_=ps[:, 32 * h : 32 * h + 32]
                        )

        # ---- Q.T projection: qt[b] [128, 10, 1024] ----
        qt_r = [
            qa.tile([P, NCO, S], BF16, tag="qa", name=f"qt{b}") for b in range(BPC)
        ]
        for bi, (cc, w) in enumerate(CBLOCKS):
            if bi == 0:
                blk = wq0
            else:
                blk = wblk.tile([P, NCI_Q, 512], BF16, tag="wblk", name=f"wq{bi}")
                nc.sync.dma_start(
                    out=blk[:, :, :w],
                    in_=wqt_d.ap()[:, cc : cc + w].rearrange("(ci p) c -> p ci c", p=P),
                )
            for b in range(BPC):
                for col in range(0, w, P):
                    co = (cc + col) // P
                    for st in range(NST):
                        ps = ps_qo.tile([P, 512], F32, tag="ps")
                        for ci in range(NCI_Q):
                            nc.tensor.matmul(
                                ps,
                                blk[:, ci, col : col + P],
                                xt_r[b][:, ci, st * 512 : st * 512 + 512],
                                start=(ci == 0),
                                stop=(ci == NCI_Q - 1),
                            )
                        nc.vector.tensor_copy(
                            out=qt_r[b][:, co, st * 512 : st * 512 + 512], in_=ps
                        )

        # ---- attention per (batch, head, s-chunk) -> at[b] [128, 10, 1024] ----
        at_r = [
            qa.tile([P, NCO, S], BF16, tag="qa", name=f"at{b}") for b in range(BPC)
        ]
        # ---- attention (st outer, h inner) interleaved with O projection ----
        # each (b, st) half's O-proj tiles are emitted right after the
        # attention half that writes their at-range, so the in-order PE queue
        # fills attention's DVE/ACT chain latency with O-proj matmuls
        wo_blocks = []
        for bi, (cc, w) in enumerate(CBLOCKS):
            oblk = wblk.tile([P, NCI_Q, 512], BF16, tag="wblk", name=f"wo{bi}")
            nc.sync.dma_start(
                out=oblk[:, :, :w],
                in_=wot_d.ap()[:, cc : cc + w].rearrange("(ci p) c -> p ci c", p=P),
            )
            wo_blocks.append(oblk)

        def attn_iter(b, h, st):
            bs = slice(b * SENC, (b + 1) * SENC)
            sl = slice(st * 512, st * 512 + 512)
            pss = ps_s.tile([SENC, 512], F32, tag="ps")
            nc.tensor.matmul(
                pss, kt[h][:, bs], qt_r[b][:, h, sl], start=True, stop=False
            )
            nc.tensor.matmul(
                pss,
                ktm[h][:, bs],
                qt_r[b][:, 8 + h // 4, sl],
                start=False,
                stop=True,
            )
            exps = expp.tile([SENC, 512], BF16, tag="exps")
            nc.scalar.activation(
                out=exps, in_=pss, func=AF.Exp, scale=ATTN_SCALE
            )
            # ones col + 32-channel remainder: row 0 = sumexp
            pav2 = ps_av2.tile([64, 512], F32, tag="av2")
            nc.tensor.matmul(pav2, v2e[b][h], exps, start=True, stop=True)
            rec = smallp.tile([1, 512], F32, tag="rec")
            nc.vector.reciprocal_approx_fast(out=rec, in_=pav2[0:1, :])
            bc = smallp.tile([P, 512], F32, tag="bc")
            nc.gpsimd.partition_broadcast(bc, rec)
            # head-major 128 channels -> tile h at offset 0, no shift
            pav = ps_av.tile([P, 512], F32, tag="av")
            nc.tensor.matmul(
                pav,
                v_nat[b][:, h * P : (h + 1) * P],
                exps,
                start=True,
                stop=True,
            )
            nc.vector.scalar_tensor_tensor(
                out=at_r[b][:, h, sl],
                in0=pav,
                scalar=1.0,
                in1=bc,
                op0=MULT,
                op1=MULT,
            )
            avt = smallp.tile([64, 512], BF16, tag="avt")
            nc.vector.scalar_tensor_tensor(
                out=avt[32:64, :],
                in0=pav2[32:64, :],
                scalar=1.0,
                in1=bc[32:64, :],
                op0=MULT,
                op1=MULT,
            )
            off = 32 * (h % 4)
            nc.sync.dma_start(
                out=at_r[b][off : off + 32, 8 + h // 4, sl], in_=avt[32:64, :]
            )


        def oproj_tile(b, sc):
            for bi, (cc, w) in enumerate(CBLOCKS):
                blk = wo_blocks[bi]
                ps = ps_qo.tile([P, 512], F32, tag="ps")
                for ci in range(NCI_Q):
                    nc.tensor.matmul(
                        ps[:, :w],
                        at_r[b][:, ci, sc * P : (sc + 1) * P],
                        blk[:, ci, :w],
                        start=(ci == 0),
                        stop=(ci == NCI_Q - 1),
                    )
                ot = ostag.tile([P, 512], F32, tag="ot")
                nc.scalar.copy(out=ot[:, :w], in_=ps[:, :w])
                nc.sync.dma_start(
                    out=out_d.ap()[b, sc * P : (sc + 1) * P, cc : cc + w],
                    in_=ot[:, :w],
                )

        for b in range(BPC):
            for h in range(H):
                for st in range(NST):
                    attn_iter(b, h, st)
        for b in range(BPC):
            for sc in range(S // P):
                oproj_tile(b, sc)

    nc.compile()
    return nc


_NC_CACHE = []


def _get_nc():
    if not _NC_CACHE:
        _NC_CACHE.append(build())
    return _NC_CACHE[0]


def make_in_maps(hidden_states, encoder_hidden_states, Wq, Wk, Wv, Wo,
                 q_down, q_up, k_down, k_up, v_down, v_up, o_down, o_up):
    bf = ml_dtypes.bfloat16
    wq = Wq.astype(np.float64) + q_up.astype(np.float64) @ q_down.astype(np.float64)
    wk = Wk.astype(np.float64) + k_up.astype(np.float64) @ k_down.astype(np.float64)
    wv = Wv.astype(np.float64) + v_up.astype(np.float64) @ v_down.astype(np.float64)
    wo = Wo.astype(np.float64) + o_up.astype(np.float64) @ o_down.astype(np.float64)
    # permute QKV output channels / Wo input channels to head-tile layout
    wqt = np.ascontiguousarray(wq[PERM, :].T.astype(bf))
    wkt = np.ascontiguousarray(wk[PERM, :].T.astype(bf))
    wvt = np.ascontiguousarray(wv[PERM, :].T.astype(bf))
    wot = np.ascontiguousarray(wo[:, PERM].T.astype(bf))

    in_maps = []
    for c in range(NCORES):
        hs = hidden_states[c * BPC : (c + 1) * BPC]  # [2, S, C]
        xt = np.ascontiguousarray(hs.transpose(0, 2, 1)).astype(bf)
        enc = encoder_hidden_states[c * BPC : (c + 1) * BPC]  # [2, 77, 1024]
        et = np.zeros((CENC, EPAD), bf)
        for b in range(BPC):
            et[:, b * SENC : (b + 1) * SENC] = enc[b].T.astype(bf)
        in_maps.append(
            {"xt": xt, "et": et, "wqt": wqt, "wkt": wkt, "wvt": wvt, "wot": wot}
        )
    return in_maps


def kernel(hidden_states, encoder_hidden_states, Wq, Wk, Wv, Wo, bo,
           q_down, q_up, k_down, k_up, v_down, v_up, o_down, o_up):
    nc = _get_nc()
    in_maps = make_in_maps(
        hidden_states, encoder_hidden_states, Wq, Wk, Wv, Wo,
        q_down, q_up, k_down, k_up, v_down, v_up, o_down, o_up,
    )
    res = run_bass_kernel_spmd(nc, in_maps, list(range(NCORES)))
    out = np.concatenate([res.results[c]["out"] for c in range(NCORES)], axis=0)
    out = out + bo.astype(np.float32)[None, None, :]
    return out.astype(np.float32)
